# revision 2
# baseline (speedup 1.0000x reference)
"""Cross-attention kernel for TRN2 (8 NeuronCores, data-parallel over batch).

Problem (per batch element b):
    s[e,t] = sum_d enc[b,e,d] * dec[b,t,d]
    a      = softmax(s, axis=e)
    out[b,t,d] = sum_e a[e,t] * enc[b,e,d]

Per-core layout (B=8 -> one batch element per core):
  - mm1 computes s in [t_block=128, e] layout: lhsT = decT (d-major), rhs =
    encT (d-major); contraction over d on the PE partition axis.
  - softmax along the free axis: DVE reduce_max(negate) -> ACT exp with
    per-partition bias and accumulated row sum Z.
  - p is transposed 128x128-wise on the PE (identity matmul) so the second
    matmul can contract over e; mm2: lhsT = pT, rhs = enc (natural layout).
  - 1/Z is applied during PSUM evacuation of mm2 (per-partition scalar mul).

Host side transposes enc/dec once (numpy) so the device never transposes
inputs.
"""

import numpy as np

import concourse.bass as bass
import concourse.tile as tile
from concourse import masks, mybir
from concourse.bass_utils import run_bass_kernel_spmd

F32 = mybir.dt.float32
F32R = mybir.dt.float32r

B, S_ENC, S_DEC, D = 8, 2048, 2048, 512
N_CORES = 8

# Matmul input precision knobs (F32 = exact, F32R = ~1e-4, 4x faster rows)
MM1_DT = F32
MM2_DT = F32


def _split_multi_waits(nc):
    """This walrus build rejects any instruction with >1 sync wait. Hoist
    surplus waits onto single-wait same-engine NOPs placed just before."""
    for f in nc.m.functions:
        for bb in f.blocks:
            new_list = []
            changed = False
            for inst in bb.instructions:
                si = inst.sync_info
                waits = list(si.on_wait) if si and si.on_wait else []
                if len(waits) > 1:
                    changed = True
                    for w in waits[:-1]:
                        nop = mybir.InstNoOp(
                            name=nc.get_next_instruction_name(),
                            engine=inst.engine,
                            sync_info=mybir.SyncInfo(on_wait=[w], on_update=[]),
                            bass_nofuse=True,
                        )
                        nc.register_instruction(nop, overwrite=True)
                        new_list.append(nop)
                    si.on_wait = waits[-1:]
                new_list.append(inst)
            if changed:
                bb.instructions = new_list


def attention_body(tc, out, encT, decT, enc, E, T, Dd, mm1_dt, mm2_dt):
    nc = tc.nc
    KD = Dd // 128   # d-tiles (contraction of mm1)
    NE = E // 512    # e-chunks of mm1 output (psum bank-sized)
    JT = E // 128    # e-tiles (contraction of mm2 / transposes)
    TB = T // 128    # t row-blocks
    Exp = mybir.ActivationFunctionType.Exp
    X = mybir.AxisListType.X

    with (
        tc.tile_pool(name="resident", bufs=1) as res_pool,
        tc.tile_pool(name="staging", bufs=2) as staging,
        tc.tile_pool(name="work", bufs=2) as work,
        tc.tile_pool(name="ps_s", bufs=1, space="PSUM") as ps_s,
        tc.tile_pool(name="ps_t", bufs=2, space="PSUM") as ps_t,
        tc.tile_pool(name="ps_c", bufs=2, space="PSUM") as ps_c,
    ):
        encTt = res_pool.tile([128, KD, E], mm1_dt)
        decTt = res_pool.tile([128, KD, T], mm1_dt)
        encS = res_pool.tile([128, JT, Dd], mm2_dt)
        ident = res_pool.tile([128, 128], mm2_dt)

        if mm1_dt == F32:
            nc.gpsimd.dma_start(encTt[:], encT.rearrange("(k p) e -> p k e", p=128))
            nc.gpsimd.dma_start(decTt[:], decT.rearrange("(k p) t -> p k t", p=128))
        else:
            for k in range(KD):
                st = staging.tile([128, E], F32, tag="stage1")
                nc.gpsimd.dma_start(st[:], encT[k * 128:(k + 1) * 128, :])
                nc.vector.tensor_copy(encTt[:, k, :], st[:])
            for k in range(KD):
                st = staging.tile([128, T], F32, tag="stage1")
                nc.gpsimd.dma_start(st[:], decT[k * 128:(k + 1) * 128, :])
                nc.vector.tensor_copy(decTt[:, k, :], st[:])

        if mm2_dt == F32:
            nc.gpsimd.dma_start(encS[:], enc.rearrange("(j p) d -> p j d", p=128))
            masks.make_identity(nc, ident[:])
        else:
            for j in range(JT):
                st = staging.tile([128, Dd], F32, tag="stage2")
                nc.gpsimd.dma_start(st[:], enc[j * 128:(j + 1) * 128, :])
                nc.vector.tensor_copy(encS[:, j, :], st[:])
            identf = res_pool.tile([128, 128], F32)
            masks.make_identity(nc, identf[:])
            nc.vector.tensor_copy(ident[:], identf[:])

        state = None
        for tb in range(TB + 1):
            cur = None
            if tb < TB:
                # mm1: s[t_block, e] accumulated over d
                psum_s = ps_s.tile([128, E], F32, tag="s")
                for k in range(KD):
                    lhsT = decTt[:, k, tb * 128:(tb + 1) * 128]
                    for n in range(NE):
                        nc.tensor.matmul(
                            psum_s[:, n * 512:(n + 1) * 512],
                            lhsT,
                            encTt[:, k, n * 512:(n + 1) * 512],
                            start=(k == 0),
                            stop=(k == KD - 1),
                        )
                # softmax along free axis
                negm = work.tile([128, 1], F32, tag="negm")
                nc.vector.reduce_max(out=negm[:], in_=psum_s[:], axis=X, negate=True)
                p = work.tile([128, E], mm2_dt, tag="p")
                z = work.tile([128, 1], F32, tag="z")
                nc.scalar.activation(out=p[:], in_=psum_s[:], func=Exp,
                                     bias=negm[:], scale=1.0, accum_out=z[:])
                rz = work.tile([128, 1], F32, tag="rz")
                nc.vector.reciprocal(rz[:], z[:])
                cur = (p, rz, tb)

            if state is not None:
                pp, rz, tbp = state
                pT = work.tile([128, JT, 128], mm2_dt, tag="pT")
                for j in range(JT):
                    pst = ps_t.tile([128, 128], mm2_dt, tag="pt")
                    nc.tensor.transpose(pst[:], pp[:, j * 128:(j + 1) * 128], ident[:])
                    nc.any.tensor_copy(pT[:, j, :], pst[:])
                psum_c = ps_c.tile([128, Dd], F32, tag="c")
                for j in range(JT):
                    nc.tensor.matmul(psum_c[:], pT[:, j, :], encS[:, j, :],
                                     start=(j == 0), stop=(j == JT - 1))
                c = work.tile([128, Dd], F32, tag="c_sb")
                nc.vector.tensor_scalar_mul(c[:], psum_c[:], rz[:])
                nc.gpsimd.dma_start(out[tbp * 128:(tbp + 1) * 128, :], c[:])

            state = cur


def build(E=S_ENC, T=S_DEC, Dd=D, mm1_dt=MM1_DT, mm2_dt=MM2_DT):
    nc = bass.Bass("TRN2", target_bir_lowering=False, debug=False)
    encT = nc.dram_tensor("encT", [Dd, E], F32, kind="ExternalInput").ap()
    decT = nc.dram_tensor("decT", [Dd, T], F32, kind="ExternalInput").ap()
    enc = nc.dram_tensor("enc", [E, Dd], F32, kind="ExternalInput").ap()
    out = nc.dram_tensor("out", [T, Dd], F32, kind="ExternalOutput").ap()
    with tile.TileContext(nc) as tc:
        attention_body(tc, out, encT, decT, enc, E, T, Dd, mm1_dt, mm2_dt)
    _split_multi_waits(nc)
    return nc


def make_in_maps(enc_output, dec_output):
    enc_output = np.asarray(enc_output, dtype=np.float32)
    dec_output = np.asarray(dec_output, dtype=np.float32)
    in_maps = []
    for b in range(B):
        in_maps.append({
            "encT": np.ascontiguousarray(enc_output[b].T),
            "decT": np.ascontiguousarray(dec_output[b].T),
            "enc": np.ascontiguousarray(enc_output[b]),
        })
    return in_maps


_nc_cache = {}


def _get_nc():
    key = (MM1_DT, MM2_DT)
    if key not in _nc_cache:
        _nc_cache[key] = build()
    return _nc_cache[key]


def kernel(enc_output, dec_output):
    nc = _get_nc()
    in_maps = make_in_maps(enc_output, dec_output)
    res = run_bass_kernel_spmd(nc, in_maps, list(range(N_CORES)))
    return np.stack([res.results[b]["out"] for b in range(B)])


# revision 3
# speedup vs baseline: 2.1036x; 2.1036x over previous
"""Cross-attention kernel for TRN2 (8 NeuronCores, data-parallel over batch).

Problem (per batch element b):
    s[e,t] = sum_d enc[b,e,d] * dec[b,t,d]
    a      = softmax(s, axis=e)
    out[b,t,d] = sum_e a[e,t] * enc[b,e,d]

Per-core layout (B=8 -> one batch element per core):
  - mm1 computes s in [t_block=128, e] layout: lhsT = decT (d-major), rhs =
    encT (d-major); contraction over d on the PE partition axis.
  - softmax along the free axis: DVE reduce_max(negate) -> ACT exp with
    per-partition bias and accumulated row sum Z.
  - p is transposed 128x128-wise on the PE (identity matmul) so the second
    matmul can contract over e; mm2: lhsT = pT, rhs = enc (natural layout).
  - 1/Z is applied during PSUM evacuation of mm2 (per-partition scalar mul).

Host side transposes enc/dec once (numpy) so the device never transposes
inputs.
"""

import numpy as np

import concourse.bass as bass
import concourse.tile as tile
from concourse import masks, mybir
from concourse.bass_utils import run_bass_kernel_spmd

F32 = mybir.dt.float32
F32R = mybir.dt.float32r

B, S_ENC, S_DEC, D = 8, 2048, 2048, 512
N_CORES = 8

# Matmul input precision knobs (F32 = exact, F32R = ~1e-4, 4x faster rows)
MM1_DT = F32R
MM2_DT = F32R


def _split_multi_waits(nc):
    """This walrus build rejects any instruction with >1 sync wait. Hoist
    surplus waits onto single-wait same-engine NOPs placed just before."""
    for f in nc.m.functions:
        for bb in f.blocks:
            new_list = []
            changed = False
            for inst in bb.instructions:
                si = inst.sync_info
                waits = list(si.on_wait) if si and si.on_wait else []
                if len(waits) > 1:
                    changed = True
                    for w in waits[:-1]:
                        nop = mybir.InstNoOp(
                            name=nc.get_next_instruction_name(),
                            engine=inst.engine,
                            sync_info=mybir.SyncInfo(on_wait=[w], on_update=[]),
                            bass_nofuse=True,
                        )
                        nc.register_instruction(nop, overwrite=True)
                        new_list.append(nop)
                    si.on_wait = waits[-1:]
                new_list.append(inst)
            if changed:
                bb.instructions = new_list


def attention_body(tc, out, encT, decT, enc, E, T, Dd, mm1_dt, mm2_dt):
    nc = tc.nc
    KD = Dd // 128   # d-tiles (contraction of mm1)
    NE = E // 512    # e-chunks of mm1 output (psum bank-sized)
    JT = E // 128    # e-tiles (contraction of mm2 / transposes)
    TB = T // 128    # t row-blocks
    Exp = mybir.ActivationFunctionType.Exp
    X = mybir.AxisListType.X

    with (
        tc.tile_pool(name="resident", bufs=1) as res_pool,
        tc.tile_pool(name="staging", bufs=2) as staging,
        tc.tile_pool(name="work", bufs=2) as work,
        tc.tile_pool(name="ps_s", bufs=1, space="PSUM") as ps_s,
        tc.tile_pool(name="ps_t", bufs=2, space="PSUM") as ps_t,
        tc.tile_pool(name="ps_c", bufs=2, space="PSUM") as ps_c,
    ):
        encTt = res_pool.tile([128, KD, E], mm1_dt)
        decTt = res_pool.tile([128, KD, T], mm1_dt)
        encS = res_pool.tile([128, JT, Dd], mm2_dt)
        ident = res_pool.tile([128, 128], mm2_dt)

        if mm1_dt == F32:
            nc.gpsimd.dma_start(encTt[:], encT.rearrange("(k p) e -> p k e", p=128))
            nc.gpsimd.dma_start(decTt[:], decT.rearrange("(k p) t -> p k t", p=128))
        else:
            for k in range(KD):
                st = staging.tile([128, E], F32, tag="stage1")
                nc.gpsimd.dma_start(st[:], encT[k * 128:(k + 1) * 128, :])
                nc.vector.tensor_copy(encTt[:, k, :], st[:])
            for k in range(KD):
                st = staging.tile([128, T], F32, tag="stage1")
                nc.gpsimd.dma_start(st[:], decT[k * 128:(k + 1) * 128, :])
                nc.vector.tensor_copy(decTt[:, k, :], st[:])

        if mm2_dt == F32:
            nc.gpsimd.dma_start(encS[:], enc.rearrange("(j p) d -> p j d", p=128))
            masks.make_identity(nc, ident[:])
        else:
            for j in range(JT):
                st = staging.tile([128, Dd], F32, tag="stage2")
                nc.gpsimd.dma_start(st[:], enc[j * 128:(j + 1) * 128, :])
                nc.vector.tensor_copy(encS[:, j, :], st[:])
            identf = res_pool.tile([128, 128], F32)
            masks.make_identity(nc, identf[:])
            nc.vector.tensor_copy(ident[:], identf[:])

        state = None
        for tb in range(TB + 1):
            cur = None
            if tb < TB:
                # mm1: s[t_block, e] accumulated over d
                psum_s = ps_s.tile([128, E], F32, tag="s")
                for k in range(KD):
                    lhsT = decTt[:, k, tb * 128:(tb + 1) * 128]
                    for n in range(NE):
                        nc.tensor.matmul(
                            psum_s[:, n * 512:(n + 1) * 512],
                            lhsT,
                            encTt[:, k, n * 512:(n + 1) * 512],
                            start=(k == 0),
                            stop=(k == KD - 1),
                        )
                # softmax along free axis
                negm = work.tile([128, 1], F32, tag="negm")
                nc.vector.reduce_max(out=negm[:], in_=psum_s[:], axis=X, negate=True)
                p = work.tile([128, E], mm2_dt, tag="p")
                z = work.tile([128, 1], F32, tag="z")
                nc.scalar.activation(out=p[:], in_=psum_s[:], func=Exp,
                                     bias=negm[:], scale=1.0, accum_out=z[:])
                rz = work.tile([128, 1], F32, tag="rz")
                nc.vector.reciprocal(rz[:], z[:])
                cur = (p, rz, tb)

            if state is not None:
                pp, rz, tbp = state
                pT = work.tile([128, JT, 128], mm2_dt, tag="pT")
                for j in range(JT):
                    pst = ps_t.tile([128, 128], mm2_dt, tag="pt")
                    nc.tensor.transpose(pst[:], pp[:, j * 128:(j + 1) * 128], ident[:])
                    nc.any.tensor_copy(pT[:, j, :], pst[:])
                psum_c = ps_c.tile([128, Dd], F32, tag="c")
                for j in range(JT):
                    nc.tensor.matmul(psum_c[:], pT[:, j, :], encS[:, j, :],
                                     start=(j == 0), stop=(j == JT - 1))
                c = work.tile([128, Dd], F32, tag="c_sb")
                nc.vector.tensor_scalar_mul(c[:], psum_c[:], rz[:])
                nc.gpsimd.dma_start(out[tbp * 128:(tbp + 1) * 128, :], c[:])

            state = cur


def build(E=S_ENC, T=S_DEC, Dd=D, mm1_dt=MM1_DT, mm2_dt=MM2_DT):
    nc = bass.Bass("TRN2", target_bir_lowering=False, debug=False)
    encT = nc.dram_tensor("encT", [Dd, E], F32, kind="ExternalInput").ap()
    decT = nc.dram_tensor("decT", [Dd, T], F32, kind="ExternalInput").ap()
    enc = nc.dram_tensor("enc", [E, Dd], F32, kind="ExternalInput").ap()
    out = nc.dram_tensor("out", [T, Dd], F32, kind="ExternalOutput").ap()
    with tile.TileContext(nc) as tc:
        attention_body(tc, out, encT, decT, enc, E, T, Dd, mm1_dt, mm2_dt)
    _split_multi_waits(nc)
    return nc


def make_in_maps(enc_output, dec_output):
    enc_output = np.asarray(enc_output, dtype=np.float32)
    dec_output = np.asarray(dec_output, dtype=np.float32)
    in_maps = []
    for b in range(B):
        in_maps.append({
            "encT": np.ascontiguousarray(enc_output[b].T),
            "decT": np.ascontiguousarray(dec_output[b].T),
            "enc": np.ascontiguousarray(enc_output[b]),
        })
    return in_maps


_nc_cache = {}


def _get_nc():
    key = (MM1_DT, MM2_DT)
    if key not in _nc_cache:
        _nc_cache[key] = build()
    return _nc_cache[key]


def kernel(enc_output, dec_output):
    nc = _get_nc()
    in_maps = make_in_maps(enc_output, dec_output)
    res = run_bass_kernel_spmd(nc, in_maps, list(range(N_CORES)))
    return np.stack([res.results[b]["out"] for b in range(B)])
